# revision 1
# baseline (speedup 1.0000x reference)
"""Trainium2 Bass kernel for CapLayer2 (1x1-conv capsule layer with dynamic routing).

Sharding: data-parallel over batch — 8 batches per core on 8 NeuronCores.

Per-core design (2 waves x 4 batches):
  - The 1x1 conv produces BOTH pred layouts on TensorE in float32r:
      predT [i-part, o]  (for the s matmuls, contraction over i=1024)
      pred  [o-part, i]  (for the delta matmuls, contraction over o=320)
    The conv bias is folded into the evictions: a DVE tensor-add against a
    partition-broadcast bias tile for predT, and the per-partition bias
    operand of the ScalarE activation for pred.
  - Routing state b/c lives in [i-part, (batch, itile, j)] layout so the
    softmax over j (J=10) is a free-dim grouped reduction; softmax runs
    per batch so each batch's s matmuls start as soon as its own delta
    transposes land (batch-level pipelining).
  - s/delta matmuls use per-batch [10, N] PSUM tiles at base partition 0
    (column tiling is illegal for 4-byte dtypes), so squash norms are
    natural per-partition accumulators (Square with accum_out).
  - sqrt is computed as exp(0.5*ln) and get_activation_tables is pinned to
    natural_log_exp_and_others so the ACT engine never reloads its table.
  - delta [10, 1024] rows are PE-transposed back to [i-part, j] in 128-col
    blocks packed into one PSUM tile, giving a single [128, 80] DVE add
    into b per batch-iteration.
"""

import numpy as np
from contextlib import ExitStack

import concourse.bacc as bacc
import concourse.bass as bass
import concourse.hw_specs as hw_specs

# Force every activation onto the one table that contains all functions this
# kernel uses (Copy/Identity/Exp/Ln/Square) so the ACT engine loads its
# function table exactly once instead of thrashing between sets.
_ONE_TABLE = "natural_log_exp_and_others"
_orig_get_tables = hw_specs.get_activation_tables


def _pinned_tables(arch):
    tabs = _orig_get_tables(arch)
    return {k: (v if k == _ONE_TABLE else set()) for k, v in tabs.items()}


bacc.get_activation_tables = _pinned_tables
import concourse.tile as tile
from concourse import mybir
from concourse.bass_utils import run_bass_kernel_spmd

F32 = mybir.dt.float32
F32R = mybir.dt.float32r
AF = mybir.ActivationFunctionType
OP = mybir.AluOpType

N_CORES = 8
BS = 64
C_IN = 256
J = 10
D = 32
O = J * D          # 320
I = 1024           # 32*32 pixels
ROUTE_NUM = 3
B_PER_CORE = BS // N_CORES   # 8
WAVE = 4
N_WAVES = B_PER_CORE // WAVE
N_IT = I // 128    # 8
N_KT = C_IN // 128 # 2
N_OT = 3           # o tiles: 128, 128, 64


def r(ap):
    return ap.bitcast(F32R)


def strip_gather(t, kw=128):
    """[kw, 128] tile -> [kw, WAVE, J] AP selecting cols 32*b+j."""
    return bass.AP(tensor=t.tensor, offset=t.offset, ap=[list(t.ap[0]), [32, WAVE], [1, J]])[:kw]


def build_kernel(stage=5):
    nc = bacc.Bacc("TRN2", target_bir_lowering=False, debug=False, num_devices=1)

    x_d = nc.dram_tensor("x", [B_PER_CORE, C_IN, I], F32R, kind="ExternalInput")
    wt_d = nc.dram_tensor("wt", [C_IN, O], F32R, kind="ExternalInput")   # W.T
    wb_d = nc.dram_tensor("wb", [1, O], F32R, kind="ExternalInput")
    out_d = nc.dram_tensor("v", [B_PER_CORE, J, D], F32, kind="ExternalOutput")

    ident_np = np.eye(128, dtype=np.float32)
    bm = np.zeros((128, O), dtype=np.float32)
    for b4 in range(WAVE):
        for j in range(J):
            bm[32 * b4 + j, 32 * j:32 * j + 32] = 1.0
    ident_d = nc.inline_tensor(ident_np, name="ident")
    bmask_d = nc.inline_tensor(bm, name="bmask")
    c0_d = nc.inline_tensor(np.full((128, J), 1.0 / J, dtype=np.float32), name="c0")

    with tile.TileContext(nc) as tc:
        with ExitStack() as ctx:
            consts = ctx.enter_context(tc.tile_pool(name="consts", bufs=1))
            xpool = ctx.enter_context(tc.tile_pool(name="xp", bufs=3))
            ppool = ctx.enter_context(tc.tile_pool(name="pp", bufs=WAVE + 2))
            state = ctx.enter_context(tc.tile_pool(name="st", bufs=2))
            ps_conv = ctx.enter_context(tc.tile_pool(name="psc", bufs=2, space="PSUM"))
            ps_st = ctx.enter_context(tc.tile_pool(name="pss", bufs=1, space="PSUM"))
            ps_tp = ctx.enter_context(tc.tile_pool(name="pst", bufs=3, space="PSUM"))
            ps_dp = ctx.enter_context(tc.tile_pool(name="psd", bufs=2, space="PSUM"))

            # ---- constants ----
            wt_sb = consts.tile([128, N_KT * O], F32R)
            nc.sync.dma_start(
                out=wt_sb.rearrange("p (k o) -> p k o", o=O),
                in_=wt_d.ap().rearrange("(k p) o -> p k o", p=128),
            )
            bias_b128 = consts.tile([128, O], F32)
            wb_bc = bass.AP(
                tensor=wb_d, offset=0, ap=[[0, 128], [1, O]]
            ).bitcast(F32)
            nc.sync.dma_start(out=bias_b128, in_=wb_bc)
            routing_consts = {}

            def load_routing_consts():
                # Emitted after the first batches' x DMAs are enqueued so the
                # small/scatter transfers don't delay the startup-critical x.
                bias_col = consts.tile([128, N_OT], F32)
                for m in range(N_OT):
                    mw = 128 if m < 2 else 64
                    nc.sync.dma_start(
                        out=bias_col[0:mw, m:m + 1],
                        in_=wb_d.ap().bitcast(F32)[0:1, 128 * m:128 * m + mw],
                    )
                ident_sb = consts.tile([128, 16], F32)
                nc.sync.dma_start(out=ident_sb, in_=ident_d.ap()[:, :16])
                bmask_sb = consts.tile([128, O], F32)
                nc.sync.dma_start(out=bmask_sb, in_=bmask_d.ap())
                c0_sb = consts.tile([128, J], F32R)
                nc.sync.dma_start(out=c0_sb, in_=r(c0_d.ap()))
                routing_consts.update(
                    bias_col=bias_col, ident_sb=ident_sb, bmask_sb=bmask_sb,
                    c0_sb=c0_sb,
                )

            for wave in range(N_WAVES):
                # ======== conv: both layouts, 4 batches ========
                predT, pred = [], []
                for b in range(WAVE):
                    gb = wave * WAVE + b
                    x_sb = xpool.tile([128, N_KT * I], F32R, tag="x")
                    for k in range(N_KT):
                        nc.sync.dma_start(
                            out=x_sb[:, k * I:(k + 1) * I],
                            in_=x_d.ap()[gb][k * 128:(k + 1) * 128, :],
                        )
                    if wave == 0 and b == 0:
                        load_routing_consts()
                    bias_col = routing_consts["bias_col"]
                    ident_sb = routing_consts["ident_sb"]
                    bmask_sb = routing_consts["bmask_sb"]
                    c0_sb = routing_consts["c0_sb"]

                    pT = ppool.tile([128, N_IT * O], F32R, tag="predT")
                    for t in range(N_IT):
                        ps = ps_conv.tile([128, 512], F32, tag="cv")
                        for k in range(N_KT):
                            nc.tensor.matmul(
                                ps[:, :O],
                                r(x_sb[:, k * I + t * 128:k * I + t * 128 + 128]),
                                r(wt_sb[:, k * O:(k + 1) * O]),
                                start=(k == 0),
                                stop=(k == N_KT - 1),
                            )
                        # eviction fused with the conv-bias add
                        nc.vector.tensor_tensor(
                            pT[:, t * O:(t + 1) * O], ps[:, :O], bias_b128, OP.add
                        )
                    predT.append(pT)

                    pr = ppool.tile([128, N_OT * I], F32R, tag="pred")
                    for m in range(N_OT):
                        mw = 128 if m < 2 else 64
                        for h in range(2):
                            ps = ps_conv.tile([128, 512], F32, tag="cv")
                            for k in range(N_KT):
                                nc.tensor.matmul(
                                    ps[:mw],
                                    r(wt_sb[:, k * O + m * 128:k * O + m * 128 + mw]),
                                    r(x_sb[:, k * I + h * 512:k * I + h * 512 + 512]),
                                    start=(k == 0),
                                    stop=(k == N_KT - 1),
                                )
                            dst_pr = pr[:mw, m * I + h * 512:m * I + h * 512 + 512]
                            if (m * 2 + h) % 2 == 0:
                                nc.scalar.activation(
                                    dst_pr, ps[:mw], AF.Identity,
                                    bias=bias_col[0:mw, m:m + 1], scale=1.0,
                                )
                            else:
                                nc.vector.tensor_scalar_add(
                                    dst_pr, ps[:mw], bias_col[0:mw, m:m + 1]
                                )
                    pred.append(pr)

                # ======== routing ========
                if stage < 2:
                    for b in range(WAVE):
                        gb = wave * WAVE + b
                        dump = state.tile([128, D], F32, tag="v_cmp")
                        nc.vector.tensor_copy(dump[:J], predT[b][:J, :D])
                        nc.vector.tensor_add(dump[:J], dump[:J], pred[b][:J, :D].bitcast(F32))
                        nc.sync.dma_start(out=out_d.ap()[gb], in_=dump[:J])
                    continue
                b_sb = state.tile([128, WAVE * N_IT * J], F32, tag="b")
                c_sb = state.tile([128, WAVE * N_IT * J], F32R, tag="c")
                for it in range(ROUTE_NUM):
                    last = it == ROUTE_NUM - 1
                    V_sb = state.tile([128, N_OT * WAVE * J], F32R, tag="V")
                    for b in range(WAVE):
                        if it > 0:
                            # per-batch softmax over j (free-dim groups of 10)
                            sl = slice(b * N_IT * J, (b + 1) * N_IT * J)
                            e_sb = state.tile([128, N_IT * J], F32, tag="e")
                            nc.scalar.activation(e_sb, b_sb[:, sl], AF.Exp)
                            den = state.tile([128, N_IT], F32, tag="den")
                            nc.vector.reduce_sum(
                                den,
                                e_sb.rearrange("p (g j) -> p g j", j=J),
                                axis=mybir.AxisListType.X,
                            )
                            rden = state.tile([128, N_IT], F32, tag="rden")
                            nc.vector.reciprocal(rden, den)
                            nc.vector.tensor_tensor(
                                c_sb[:, sl].rearrange("p (g j) -> p g j", j=J),
                                e_sb.rearrange("p (g j) -> p g j", j=J),
                                rden.broadcast_to([128, N_IT, J]),
                                OP.mult,
                            )
                        # ---- s = c . predT  (contraction over i) ----
                        ps_s = ps_st.tile([128, O], F32, tag="s")
                        for t in range(N_IT):
                            lhs = (
                                c0_sb
                                if it == 0
                                else c_sb[:, (b * N_IT + t) * J:(b * N_IT + t + 1) * J]
                            )
                            nc.tensor.matmul(
                                ps_s[:J],
                                r(lhs),
                                r(predT[b][:, t * O:(t + 1) * O]),
                                start=(t == 0),
                                stop=(t == N_IT - 1),
                            )

                        # ---- squash (partitions = j) ----
                        s_m = state.tile([128, O], F32, tag="s_m")
                        nc.vector.tensor_tensor(s_m[:J], ps_s[:J], bmask_sb[:J], OP.mult)
                        sq = state.tile([128, O], F32, tag="sq")
                        ns = state.tile([128, 1], F32, tag="ns")
                        nc.scalar.activation(
                            sq[:J], s_m[:J], AF.Square, accum_out=ns[:J]
                        )
                        # sqrt(ns) = exp(0.5*ln(ns)) — keeps every ACT func
                        # in the natural_log_exp_and_others table (one load,
                        # no per-iteration table thrash)
                        lns = state.tile([128, 1], F32, tag="lns")
                        nc.scalar.activation(lns[:J], ns[:J], AF.Ln)
                        rt = state.tile([128, 1], F32, tag="rt")
                        nc.scalar.activation(rt[:J], lns[:J], AF.Exp, scale=0.5)
                        ns1 = state.tile([128, 1], F32, tag="ns1")
                        nc.vector.tensor_scalar_add(ns1[:J], ns[:J], 1.0)
                        rns1 = state.tile([128, 1], F32, tag="rns1")
                        nc.vector.reciprocal(rns1[:J], ns1[:J])
                        coeff = state.tile([128, 1], F32, tag="coeff")
                        nc.vector.tensor_tensor(coeff[:J], rt[:J], rns1[:J], OP.mult)
                        v_full = state.tile([128, O], F32, tag="v_full")
                        nc.vector.tensor_scalar_mul(v_full[:J], s_m[:J], coeff[:J])

                        if last or stage == 2:
                            v_cmp = state.tile([128, D], F32, tag="v_cmp")
                            nc.vector.reduce_sum(
                                v_cmp[:J],
                                v_full[:J].rearrange("p (j d) -> p d j", j=J),
                                axis=mybir.AxisListType.X,
                            )
                            if last or (stage == 2 and it == 0):
                                gb = wave * WAVE + b
                                nc.sync.dma_start(out=out_d.ap()[gb], in_=v_cmp[:J])
                            continue

                        # ---- V: transpose v into [o-part, (k, b, j)] ----
                        ps_tv = ps_tp.tile([128, N_OT * J], F32, tag="T")
                        nc.vector.memset(ps_tv[64:, 2 * J:3 * J], 0.0)
                        for k in range(N_OT):
                            kw = 128 if k < 2 else 64
                            nc.tensor.transpose(
                                ps_tv[:kw, k * J:(k + 1) * J],
                                v_full[:J, k * 128:k * 128 + kw],
                                ident_sb[:J, :J],
                            )
                        vdst = bass.AP(
                            tensor=V_sb.tensor,
                            offset=V_sb.offset + b * J,
                            ap=[list(V_sb.ap[0]), [WAVE * J, N_OT], [1, J]],
                        )
                        nc.vector.tensor_copy(vdst, ps_tv.rearrange("p (k j) -> p k j", j=J))

                    if last or stage == 2:
                        if stage == 2:
                            break
                        continue

                    if stage == 3:
                        for b in range(WAVE):
                            gb = wave * WAVE + b
                            dmp = state.tile([128, D], F32, tag="v_cmp")
                            nc.vector.tensor_copy(dmp[:J], V_sb[:J, :D].bitcast(F32))
                            nc.sync.dma_start(out=out_d.ap()[gb], in_=dmp[:J])
                        break
                    for b in range(WAVE):
                        # ---- delta = V . pred  (contraction over o) ----
                        delta_sb = state.tile([128, I], F32, tag="delta")
                        for h in range(2):
                            ps_d = ps_dp.tile([128, 512], F32, tag="d")
                            for k in range(N_OT):
                                kw = 128 if k < 2 else 64
                                nc.tensor.matmul(
                                    ps_d[:J],
                                    r(V_sb[:kw, (k * WAVE + b) * J:(k * WAVE + b + 1) * J]),
                                    r(pred[b][:kw, k * I + h * 512:k * I + (h + 1) * 512]),
                                    start=(k == 0),
                                    stop=(k == N_OT - 1),
                                )
                            nc.scalar.copy(delta_sb[:J, h * 512:(h + 1) * 512], ps_d[:J])

                        if stage == 4:
                            gb = wave * WAVE + b
                            dmp2 = state.tile([128, D], F32, tag="v_cmp")
                            nc.vector.tensor_copy(dmp2[:J], delta_sb[:J, :D])
                            nc.sync.dma_start(out=out_d.ap()[gb], in_=dmp2[:J])
                            continue
                        # ---- transpose delta back into [i-part, j] ----
                        ps_t = ps_tp.tile([128, N_IT * J], F32, tag="T")
                        for t in range(N_IT):
                            nc.tensor.transpose(
                                ps_t[:, t * J:(t + 1) * J],
                                delta_sb[:J, t * 128:(t + 1) * 128],
                                ident_sb[:J, :J],
                            )
                        dst = b_sb[:, b * N_IT * J:(b + 1) * N_IT * J]
                        if it == 0:
                            nc.vector.tensor_copy(dst, ps_t)
                        else:
                            nc.vector.tensor_tensor(dst, ps_t, dst, OP.add)
                    if stage == 4:
                        break

    nc.compile()
    return nc


_NC_CACHE = None
LAST_RESULT = None


def kernel(x: np.ndarray, W: np.ndarray, W_b: np.ndarray) -> np.ndarray:
    global _NC_CACHE
    if _NC_CACHE is None:
        _NC_CACHE = build_kernel()
    nc = _NC_CACHE

    x = np.ascontiguousarray(x.reshape(BS, C_IN, I), dtype=np.float32)
    wt = np.ascontiguousarray(W.T, dtype=np.float32)
    wb = np.ascontiguousarray(W_b.reshape(1, O), dtype=np.float32)

    in_maps = [
        {
            "x": np.ascontiguousarray(x[c * B_PER_CORE:(c + 1) * B_PER_CORE]),
            "wt": wt,
            "wb": wb,
        }
        for c in range(N_CORES)
    ]
    import os
    trace = bool(int(os.environ.get("KERNEL_TRACE", "0")))
    res = run_bass_kernel_spmd(
        nc, in_maps, core_ids=list(range(N_CORES)), trace=trace
    )
    if trace:
        global LAST_RESULT
        LAST_RESULT = res
    out = np.concatenate([res.results[c]["v"] for c in range(N_CORES)], axis=0)
    return out.astype(np.float32)


if __name__ == "__main__":
    rng = np.random.default_rng(0)
    x = rng.standard_normal((BS, C_IN, 32, 32), dtype=np.float32)
    W = (rng.standard_normal((O, C_IN)) * 0.02).astype(np.float32)
    W_b = (rng.standard_normal((O,)) * 0.02).astype(np.float32)
    v = kernel(x=x, W=W, W_b=W_b)
    print(v.shape, v.dtype, float(np.abs(v).max()))



# revision 10
# speedup vs baseline: 2.4146x; 2.4146x over previous
"""Trainium2 Bass kernel for CapLayer2 (1x1-conv capsule layer with dynamic routing).

Sharding: data-parallel over batch - 8 batches per core on 8 NeuronCores.

Compressed-routing formulation: the [320, 1024] conv output `pred` is never
materialized.  With W_j = rows jD..(j+1)D of W and the conv bias folded in as
an augmented channel (x_aug row 256 = 1, W_aug col 256 = b):

    s[j]     = W_aug_j @ y_aug[j]      where y_aug[j, c'] = sum_i c[j,i] x_aug[c',i]
    delta[j] = u_aug[:,j] . x_aug      where u_aug = W_aug^T vmask

so per routing iteration the PE contracts against the small [c', J]-sized
quantities instead of the full 320-wide pred.  Everything runs in bf16
(inputs converted on host, halving the x DMA), with f32 PSUM accumulation.

Per-core schedule:
  - x arrives per batch (1 DMA each); PE transposes x -> xT (bf16, via PSUM)
    for the y matmuls; iteration-0 y (uniform c0 = 1/J) rides the load phase.
  - Routing state b lives in PSUM: deltaT matmuls write [i-part, (b,t,j)]
    tiles directly and iteration 1 accumulates on top (start=False), so b
    never round-trips through SBUF.
  - Softmax over j is a free-dim grouped reduction on [128, 320] halves;
    s / squash / u run batched across all 8 batches ([80, 320] tiles).
"""

import os
import numpy as np
from contextlib import ExitStack

import concourse.bacc as bacc
import concourse.bass as bass
import concourse.hw_specs as hw_specs

# Pin every activation onto the one table that contains all functions this
# kernel uses (Copy/Identity/Exp/Ln/Square) so the ACT engine loads its
# function table exactly once.
_ONE_TABLE = "natural_log_exp_and_others"
_orig_get_tables = hw_specs.get_activation_tables


def _pinned_tables(arch):
    tabs = _orig_get_tables(arch)
    return {k: (v if k == _ONE_TABLE else set()) for k, v in tabs.items()}


bacc.get_activation_tables = _pinned_tables
import concourse.tile as tile
from concourse import mybir
from concourse.bass_utils import run_bass_kernel_spmd

F32 = mybir.dt.float32
BF16 = mybir.dt.bfloat16
AF = mybir.ActivationFunctionType
OP = mybir.AluOpType

N_CORES = 8
BS = 64
C_IN = 256
J = 10
D = 32
O = J * D          # 320
I = 1024           # 32*32 pixels
ROUTE_NUM = 3
B_PER_CORE = BS // N_CORES   # 8
N_IT = I // 128    # 8 i-tiles
HB = B_PER_CORE // 2         # batches per psum half (4)


def build_kernel(dbg=False):
    nc = bacc.Bacc("TRN2", target_bir_lowering=False, debug=False, num_devices=1)
    dbg_d = nc.dram_tensor("dbg", [2, 128, 320], F32, kind="ExternalOutput") if dbg else None

    x_d = nc.dram_tensor("x", [B_PER_CORE, C_IN, I], BF16, kind="ExternalInput")
    # wt3[k] = W.T rows 128k..128(k+1); wt3[2,0] = W_b  (rest of slab 2 zero)
    wt3_d = nc.dram_tensor("wt3", [3, 128, O], BF16, kind="ExternalInput")
    # waug[m] = [W | W_b] rows 128m.. (o-padded to 384 with zeros), cols c'=257
    waug_d = nc.dram_tensor("waug", [3, 128, 257], BF16, kind="ExternalInput")
    out_d = nc.dram_tensor("v", [B_PER_CORE, J, D], F32, kind="ExternalOutput")

    # combined bf16 consts: [eye128 | ones128 | c0 (10 cols of 1/J) | csum0 row]
    cbn = np.zeros((128, 347), dtype=np.float32)
    cbn[:, 0:128] = np.eye(128)
    cbn[:, 128:256] = 1.0
    cbn[:, 256:266] = 1.0 / J
    cbn[0, 266:346] = I / J
    import ml_dtypes
    cb_d = nc.inline_tensor(cbn.astype(ml_dtypes.bfloat16), name="cb")
    bm = np.zeros((128, O), dtype=np.float32)
    for b4 in range(B_PER_CORE):
        for j in range(J):
            bm[J * b4 + j, D * j:D * j + D] = 1.0
    bmask_d = nc.inline_tensor(bm, name="bmask")

    with tile.TileContext(nc) as tc:
        with ExitStack() as ctx:
            consts = ctx.enter_context(tc.tile_pool(name="consts", bufs=1))
            xpool = ctx.enter_context(tc.tile_pool(name="xp", bufs=1))
            state = ctx.enter_context(tc.tile_pool(name="st", bufs=2))
            st3 = ctx.enter_context(tc.tile_pool(name="st3", bufs=3))
            ps_xp = ctx.enter_context(tc.tile_pool(name="psx", bufs=2, space="PSUM"))
            ps_dp = ctx.enter_context(tc.tile_pool(name="psd", bufs=1, space="PSUM"))
            ps_yp = ctx.enter_context(tc.tile_pool(name="psy", bufs=1, space="PSUM"))
            ps_sp = ctx.enter_context(tc.tile_pool(name="pss", bufs=1, space="PSUM"))
            ps_up = ctx.enter_context(tc.tile_pool(name="psu", bufs=1, space="PSUM"))
            ps_vp = ctx.enter_context(tc.tile_pool(name="psv", bufs=1, space="PSUM"))

            # ---- constants ----
            cb_sb = consts.tile([128, 347], BF16)
            nc.sync.dma_start(out=cb_sb, in_=cb_d.ap())
            eye = cb_sb[:, 0:128]
            ones_col = cb_sb[:, 128:129]
            ones_row = cb_sb[0:1, 128:256]
            c0_sb = cb_sb[:, 256:266]
            csum0 = cb_sb[0:1, 266:346]

            def load_more_consts():
                wt_sb0 = consts.tile([128, 3 * O], BF16)
                nc.sync.dma_start(
                    out=wt_sb0.rearrange("p (k o) -> p k o", o=O),
                    in_=wt3_d.ap().rearrange("k p o -> p k o"),
                )
                waug_sb0 = consts.tile([128, 3 * 257], BF16)
                nc.sync.dma_start(
                    out=waug_sb0.rearrange("p (m c) -> p m c", c=257),
                    in_=waug_d.ap().rearrange("m p c -> p m c"),
                )
                bmask_sb0 = consts.tile([128, O], F32)
                nc.sync.dma_start(out=bmask_sb0, in_=bmask_d.ap())
                return wt_sb0, waug_sb0, bmask_sb0

            # persistent per-half routing-logit psum: cols = (b%4, t, j)
            psd = [ps_dp.tile([128, HB * N_IT * J], F32, tag=f"d{h}", name=f"psd{h}") for h in range(2)]
            # SBUF accumulator for b across iterations (psum groups can't reopen)
            b_sb = [state.tile([128, HB * N_IT * J], F32, tag=f"b{h}", name=f"bsb{h}") for h in range(2)]

            x_sb, xT_sb = [], []
            wt_sb = waug_sb = bmask_sb = None
            # iteration-0 y psum (written per batch during the load phase)
            ps_y = ps_yp.tile([128, 3 * 80], F32, tag="y")
            for b in range(B_PER_CORE):
                xb = xpool.tile([128, 2 * I], BF16, tag=f"x{b}")
                nc.sync.dma_start(
                    out=xb.rearrange("p (k i) -> p k i", i=I),
                    in_=x_d.ap()[b].rearrange("(k p) i -> p k i", p=128),
                )
                x_sb.append(xb)
                if b == 0:
                    wt_sb, waug_sb, bmask_sb = load_more_consts()
                xt = xpool.tile([128, N_IT * C_IN], BF16, tag=f"xt{b}")
                xT_sb.append(xt)
                xv = xb.rearrange("p (k i) -> p k i", i=I)
                for g in range(2):
                    ps = ps_xp.tile([128, 1024], BF16, tag="xp")
                    for tt in range(4):
                        t = g * 4 + tt
                        for k in range(2):
                            nc.tensor.transpose(
                                ps[:, (tt * 2 + k) * 128:(tt * 2 + k + 1) * 128],
                                xv[:, k, t * 128:(t + 1) * 128],
                                eye,
                            )
                    dst = xt[:, g * 1024:(g + 1) * 1024]
                    if g == 0:
                        nc.vector.tensor_copy(dst, ps)
                    else:
                        nc.scalar.copy(dst, ps)
                # iteration-0 y for this batch (c = c0 uniform; csum0 is const)
                xtv = xt.rearrange("p (t c) -> p t c", c=C_IN)
                yv = ps_y.rearrange("p (k c) -> p k c", c=80)
                for k in range(2):
                    for t in range(N_IT):
                        nc.tensor.matmul(
                            yv[:, k, b * J:(b + 1) * J],
                            xtv[:, t, k * 128:k * 128 + 128],
                            c0_sb,
                            start=(t == 0),
                            stop=(t == N_IT - 1),
                        )

            wtv = wt_sb.rearrange("p (k o) -> p k o", o=O)
            wav = waug_sb.rearrange("p (m c) -> p m c", c=257)

            # ---- routing ----
            for it in range(ROUTE_NUM):
                last = it == ROUTE_NUM - 1
                if it > 0:
                    # softmax over j (free-dim groups of 10), one [128,320] half at a time
                    c_half = []
                    for h in range(2):
                        e_sb = st3.tile([128, HB * N_IT * J], F32, tag="e")
                        nc.scalar.activation(
                            e_sb, psd[h] if it == 1 else b_sb[h], AF.Exp
                        )
                        den = st3.tile([128, HB * N_IT], F32, tag="den")
                        nc.vector.reduce_sum(
                            den,
                            e_sb.rearrange("p (g j) -> p g j", j=J),
                            axis=mybir.AxisListType.X,
                        )
                        rden = st3.tile([128, HB * N_IT], F32, tag="rden")
                        nc.vector.reciprocal(rden, den)
                        ch = st3.tile([128, HB * N_IT * J], BF16, tag="c")
                        nc.vector.tensor_tensor(
                            ch.rearrange("p (g j) -> p g j", j=J),
                            e_sb.rearrange("p (g j) -> p g j", j=J),
                            rden.broadcast_to([128, HB * N_IT, J]),
                            OP.mult,
                        )
                        c_half.append(ch)
                    ps_y = ps_yp.tile([128, 3 * 80], F32, tag="y")
                    yv = ps_y.rearrange("p (k c) -> p k c", c=80)
                    for b in range(B_PER_CORE):
                        ch = c_half[b // HB].rearrange("p (q t j) -> p q t j", t=N_IT, j=J)
                        xtv = xT_sb[b].rearrange("p (t c) -> p t c", c=C_IN)
                        for k in range(2):
                            for t in range(N_IT):
                                nc.tensor.matmul(
                                    yv[:, k, b * J:(b + 1) * J],
                                    xtv[:, t, k * 128:k * 128 + 128],
                                    ch[:, b % HB, t, :],
                                    start=(t == 0),
                                    stop=(t == N_IT - 1),
                                )
                        for t in range(N_IT):
                            nc.tensor.matmul(
                                yv[0:1, 2, b * J:(b + 1) * J],
                                ones_col,
                                ch[:, b % HB, t, :],
                                start=(t == 0),
                                stop=(t == N_IT - 1),
                            )
                # evict y (bf16) - slice 2 only has partition 0 written
                y_sb = state.tile([128, 3 * 80], BF16, tag="ysb")
                if it == 0:
                    nc.vector.tensor_copy(y_sb[:, 0:160], ps_y[:, 0:160])
                else:
                    nc.vector.tensor_copy(y_sb[:, 0:160], ps_y[:, 0:160])
                    nc.vector.tensor_copy(y_sb[0:1, 160:240], ps_y[0:1, 160:240])
                yj = y_sb.rearrange("p (k c) -> p k c", c=80)

                # s = y_aug @ W_aug^T for all batches: [80, 320]
                ps_s = ps_sp.tile([128, O], F32, tag="s")
                for k in range(2):
                    nc.tensor.matmul(
                        ps_s[:80], yj[:, k, :], wtv[:, k, :],
                        start=(k == 0), stop=False,
                    )
                nc.tensor.matmul(
                    ps_s[:80],
                    csum0 if it == 0 else yj[0:1, 2, :],
                    wtv[0:1, 2, :],
                    start=False, stop=True,
                )

                # squash (partitions = (b, j))
                s_m = state.tile([128, O], F32, tag="s_m")
                nc.vector.tensor_tensor(s_m[:80], ps_s[:80], bmask_sb[:80], OP.mult)
                sq = state.tile([128, O], F32, tag="sq")
                ns = state.tile([128, 1], F32, tag="ns")
                nc.scalar.activation(sq[:80], s_m[:80], AF.Square, accum_out=ns[:80])
                lns = state.tile([128, 1], F32, tag="lns")
                nc.scalar.activation(lns[:80], ns[:80], AF.Ln)
                rt = state.tile([128, 1], F32, tag="rt")
                nc.scalar.activation(rt[:80], lns[:80], AF.Exp, scale=0.5)
                ns1 = state.tile([128, 1], F32, tag="ns1")
                nc.vector.tensor_scalar_add(ns1[:80], ns[:80], 1.0)
                rns1 = state.tile([128, 1], F32, tag="rns1")
                nc.vector.reciprocal(rns1[:80], ns1[:80])
                coeff = state.tile([128, 1], F32, tag="coeff")
                nc.vector.tensor_tensor(coeff[:80], rt[:80], rns1[:80], OP.mult)

                if last:
                    vred = state.tile([128, D], F32, tag="vred")
                    nc.vector.reduce_sum(
                        vred[:80],
                        s_m[:80].rearrange("p (j d) -> p d j", j=J),
                        axis=mybir.AxisListType.X,
                    )
                    v_out = state.tile([128, D], F32, tag="vout")
                    nc.vector.tensor_scalar_mul(v_out[:80], vred[:80], coeff[:80])
                    nc.sync.dma_start(
                        out=out_d.ap().rearrange("b j d -> (b j) d"),
                        in_=v_out[:80],
                    )
                    continue

                v_full = state.tile([128, O], BF16, tag="vf")
                nc.vector.tensor_scalar_mul(v_full[:80], s_m[:80], coeff[:80])

                # vmask = v_full^T: [o-part, (b, j)] bf16
                ps_vT = ps_vp.tile([128, 3 * 80], BF16, tag="vT")
                for k in range(3):
                    kw = 128 if k < 2 else 64
                    nc.tensor.transpose(
                        ps_vT[0:kw, k * 80:(k + 1) * 80],
                        v_full[0:80, k * 128:k * 128 + kw],
                        eye[0:80, 0:80],
                    )
                vmask = state.tile([128, 3 * 80], BF16, tag="vm")
                nc.vector.tensor_copy(vmask[:, 0:160], ps_vT[:, 0:160])
                nc.vector.tensor_copy(vmask[0:64, 160:240], ps_vT[0:64, 160:240])
                vmv = vmask.rearrange("p (m c) -> p m c", c=80)

                # u_aug = W_aug^T @ vmask : [c'-tiles, (b, j)]
                ps_u = ps_up.tile([128, 3 * 80], F32, tag="u")
                uv = ps_u.rearrange("p (ct c) -> p ct c", c=80)
                for ct in range(3):
                    cw = 128 if ct < 2 else 1
                    for m in range(3):
                        mw = 128 if m < 2 else 64
                        nc.tensor.matmul(
                            uv[0:cw, ct, :],
                            wav[0:mw, m, ct * 128:ct * 128 + cw],
                            vmv[0:mw, m, :],
                            start=(m == 0), stop=(m == 2),
                        )
                u_sb = state.tile([128, 3 * 80], BF16, tag="usb")
                nc.vector.tensor_copy(u_sb[:, 0:160], ps_u[:, 0:160])
                nc.scalar.copy(u_sb[0:1, 160:240], ps_u[0:1, 160:240])
                uj = u_sb.rearrange("p (k c) -> p k c", c=80)

                # deltaT: [i-part, (b%4, t, j)] accumulated into psd halves
                for b in range(B_PER_CORE):
                    pd = psd[b // HB].rearrange("p (q t j) -> p q t j", t=N_IT, j=J)
                    xv = x_sb[b].rearrange("p (k i) -> p k i", i=I)
                    for t in range(N_IT):
                        dst = pd[:, b % HB, t, :]
                        for k in range(2):
                            nc.tensor.matmul(
                                dst,
                                xv[:, k, t * 128:(t + 1) * 128],
                                uj[:, k, b * J:(b + 1) * J],
                                start=(k == 0), stop=False,
                            )
                        nc.tensor.matmul(
                            dst,
                            ones_row,
                            uj[0:1, 2, b * J:(b + 1) * J],
                            start=False, stop=True,
                        )
                # accumulate b in SBUF (softmax it=1 reads psd directly)
                for h in range(2):
                    if it == 0:
                        nc.vector.tensor_copy(b_sb[h], psd[h])
                    else:
                        nc.vector.tensor_tensor(b_sb[h], psd[h], b_sb[h], OP.add)
                if dbg and it == ROUTE_NUM - 2:
                    for h in range(2):
                        dtile = state.tile([128, 320], F32, tag=f"dbg{h}", name=f"dbg{h}")
                        nc.vector.tensor_copy(dtile, b_sb[h])
                        nc.sync.dma_start(out=dbg_d.ap()[h], in_=dtile)

    nc.compile()
    return nc


_NC_CACHE = None
LAST_RESULT = None


def kernel(x: np.ndarray, W: np.ndarray, W_b: np.ndarray) -> np.ndarray:
    global _NC_CACHE
    if _NC_CACHE is None:
        _NC_CACHE = build_kernel()
    nc = _NC_CACHE

    import ml_dtypes
    bf = ml_dtypes.bfloat16
    xb = np.ascontiguousarray(x.reshape(BS, C_IN, I)).astype(bf)
    wtT = W.astype(np.float32).T  # [C, O]
    wt3 = np.zeros((3, 128, O), dtype=np.float32)
    wt3[0] = wtT[0:128]
    wt3[1] = wtT[128:256]
    wt3[2, 0] = W_b
    waug = np.zeros((3, 128, 257), dtype=np.float32)
    wfull = np.concatenate([W.astype(np.float32), W_b[:, None].astype(np.float32)], axis=1)  # [320, 257]
    waug.reshape(384, 257)[0:O] = wfull
    wt3 = wt3.astype(bf)
    waug = waug.astype(bf)

    in_maps = [
        {
            "x": np.ascontiguousarray(xb[c * B_PER_CORE:(c + 1) * B_PER_CORE]),
            "wt3": wt3,
            "waug": waug,
        }
        for c in range(N_CORES)
    ]
    trace = bool(int(os.environ.get("KERNEL_TRACE", "0")))
    res = run_bass_kernel_spmd(
        nc, in_maps, core_ids=list(range(N_CORES)), trace=trace
    )
    if trace:
        global LAST_RESULT
        LAST_RESULT = res
    out = np.concatenate([res.results[c]["v"] for c in range(N_CORES)], axis=0)
    return out.astype(np.float32)


if __name__ == "__main__":
    rng = np.random.default_rng(0)
    x = rng.standard_normal((BS, C_IN, 32, 32), dtype=np.float32)
    W = (rng.standard_normal((O, C_IN)) * 0.02).astype(np.float32)
    W_b = (rng.standard_normal((O,)) * 0.02).astype(np.float32)
    v = kernel(x=x, W=W, W_b=W_b)
    print(v.shape, v.dtype, float(np.abs(v).max()))
